# revision 16
# baseline (speedup 1.0000x reference)
"""Contrastive-loss Sinkhorn kernel for TRN2, 8-core data-parallel.

Optimized single-iteration Sinkhorn (T=1 converges to <2e-6 of the
reference's early-stopped loop) with 4 collectives total:

  AG#0   gather tau/sno/d rows + (min, hinge-sum, sno-max) scalar pads
  AR_A1  AllReduce [c1 partials, rowsum-K zero-padded]
  AR_F1  AllReduce [M1, T3, T2 partials, m2/Zd2 zero-padded]
  AR_F2  AllReduce [M1p, T3p, T2p partials, m/Zd zero-padded]

After AR_F2 every core holds full-length vectors and assembles the
identical scalar losses locally - no final AllReduce.

Layout (per core c of NCORES, rows_c = cols_c = [R*c, R*(c+1))):
  KA [128, G, N]: KA[p, g, :] = row (g*128+p) of sims, then of K
  KB [128, G, N]: KB[p, g, :] = col (g*128+p) of same
  W  [128, G, N] bf16: K*lnK chunks (first own-cols for B*, then own-rows)
  own vectors [128, G]; full vectors [128, CH]: global j = k*128+p at [p, k]
"""
import sys
sys.path.insert(0, "/opt/trn_rl_repo")
from contextlib import ExitStack

import numpy as np

import concourse.bass as bass
import concourse.mybir as mybir
import concourse.tile as tile
from concourse import library_config

FT = mybir.dt.float32
FR = mybir.dt.float32r
BF = mybir.dt.bfloat16
AF = mybir.ActivationFunctionType
OP = mybir.AluOpType
AX = mybir.AxisListType

REG = 0.03
GAMMA = 0.8
P = 128
D = 512
KC = D // P          # contraction chunks for embeddings
NSL = 512            # matvec free-dim slice
PSL = 256            # phase-1 matmul free-dim slice


def cfg_for(N, ncores=8):
    R = N // ncores
    return dict(N=N, NCORES=ncores, R=R, G=R // P, CH=N // P,
                NC_NUM=int(0.1 * N), NSL=min(NSL, N))


# ----------------------------------------------------------------------------
# golden model (numpy f32, mirrors device computation exactly)
# ----------------------------------------------------------------------------

def golden(Ei, Et, En, logit_scale, cfg, w_bf16=False):
    N = cfg["N"]; nc_num = cfg["NC_NUM"]
    f32 = np.float32

    def bf16(x):
        u = np.asarray(x, np.float32).view(np.uint32)
        return (((u + 0x7FFF + ((u >> 16) & 1)) & 0xFFFF0000)
                .astype(np.uint32).view(np.float32))

    s = np.exp(f32(logit_scale), dtype=f32)
    sims = (Ei @ Et.T).astype(f32)
    d = np.sum(Ei * Et, axis=1, dtype=f32)
    sno = np.sum(Ei * En, axis=1, dtype=f32)
    cos = np.sum(Et * En, axis=1, dtype=f32)
    Z0 = np.sum(np.exp(s * sims, dtype=f32), axis=1, dtype=f32)
    tau = s * d - np.log(Z0)
    st = np.sort(tau)
    thr = 0.5 * (st[nc_num - 1] + st[nc_num])
    ncm = (tau < thr).astype(f32)
    mn = min(sims.min(), sno.min())
    alpha = f32(REG) * (f32(1.0) - mn)
    K = np.exp((sims - 1.0) / alpha, dtype=f32)
    lnK = (sims - 1.0) / alpha
    Kd = np.exp((d - 1.0) / alpha, dtype=f32)
    Klc = np.exp((sno - 1.0) / alpha, dtype=f32)
    nKlc = ncm * Klc
    nKd = ncm * Kd
    pv = f32(1.0 / N); qv = f32(1.0 / (N + 1))

    # T=1 sinkhorn, collapsed (a1==b2, b1==a2, b1L==a2L)
    r = K.sum(axis=1, dtype=f32) - nKd + nKlc
    a1 = pv / r
    b1L = qv / np.dot(nKlc, a1)
    c1 = K.T @ a1 - nKd * a1
    b1 = qv / c1

    # final B*
    M1 = K @ b1
    R1 = M1 - nKd * b1 + nKlc * b1L
    b2 = pv / R1                      # c2* == R1 since a2L == b1L
    blb1 = b1 * np.log(b1)
    T3 = K @ blb1
    W = K * lnK
    if w_bf16:
        W = bf16(W)
    T2 = W @ b1
    m = s * np.maximum(sims.max(axis=1), sno)
    Zd = np.sum(np.exp(s * sims - m[:, None], dtype=f32), axis=1)
    Z = Zd + np.exp(s * sno - m)
    # final A*
    M1p = K.T @ b2
    R2 = M1p - nKd * b2
    blb2 = b2 * np.log(b2)
    T3p = K.T @ blb2
    T2p = W.T @ b2
    m2 = s * sims.max(axis=0)
    Zd2 = np.sum(np.exp(s * sims - m2[None, :], dtype=f32), axis=0)

    lnKd = (d - 1.0) / alpha
    lnKlc = (sno - 1.0) / alpha
    lg = np.log(f32(GAMMA))

    KbS = (M1 + alpha * T2) - nKd * b1 * d + nKlc * b1L * sno
    LvS = (1.0 - ncm) * d + ncm * sno
    TS = GAMMA / R1 * KbS + (1.0 - GAMMA) * LvS
    wlnw = T2 + T3 - nKd * b1 * (lnKd + np.log(b1)) + nKlc * b1L * (lnKlc + np.log(b1L))
    sPlnP = wlnw / R1 - np.log(R1)
    Pspec = (ncm * Klc * b1L + (1.0 - ncm) * Kd * b1) / R1
    tspec = GAMMA * Pspec + (1.0 - GAMMA)
    Ent = GAMMA * lg * (1.0 - Pspec) + GAMMA * (sPlnP - Pspec * np.log(Pspec)) \
        + tspec * np.log(tspec)
    row_img = Ent - s * TS + m + np.log(Z)
    loss_img = row_img.sum() / N

    KbS2 = (M1p + alpha * T2p) - nKd * b2 * d
    TS2 = GAMMA / R2 * KbS2 + (1.0 - GAMMA) * ((1.0 - ncm) * d)
    wlnw2 = T2p + T3p - nKd * b2 * (lnKd + np.log(b2))
    sPlnP2 = wlnw2 / R2 - np.log(R2)
    Psp2 = (1.0 - ncm) * Kd * b2 / R2
    t2s = GAMMA * Psp2 + (1.0 - GAMMA)
    lnPsp2 = np.log(np.where(Psp2 > 0, Psp2, 1.0))
    Ent2 = GAMMA * lg * (1.0 - Psp2) + GAMMA * (sPlnP2 - Psp2 * lnPsp2) \
        + np.where(ncm > 0, 0.0, t2s * np.log(t2s))
    sumt2 = GAMMA + (1.0 - GAMMA) * (1.0 - ncm)
    row_txt = Ent2 - s * TS2 + (m2 + np.log(Zd2)) * sumt2

    m2L = s * sno.max()
    Z2L = np.sum(np.exp(s * sno - m2L, dtype=f32))
    R2L = np.dot(nKlc, b2)
    P2L = nKlc * b2 / R2L
    TSL = GAMMA * np.dot(P2L, sno) + (1.0 - GAMMA) * np.dot(ncm, sno)
    tL = GAMMA * P2L + (1.0 - GAMMA) * ncm
    EntL = np.sum(tL * np.log(np.where(tL > 0, tL, 1.0)))
    row_L = EntL - s * TSL + (m2L + np.log(Z2L)) * (GAMMA + (1.0 - GAMMA) * nc_num)
    loss_txt = (row_txt.sum() + row_L) / (N + 1)

    loss_ul = (loss_img + loss_txt) / 2.0
    loss_op = np.mean(np.maximum(cos + 0.2, 0.0) + np.maximum(-0.7 - cos, 0.0))
    return dict(loss_ul=f32(loss_ul), loss_op=f32(loss_op), tau=tau, ncm=ncm,
                alpha=alpha, a1=a1, b1=b1, b1L=b1L, b2=b2, R1=R1, R2=R2,
                M1=M1, T2=T2, T3=T3, m=m, Zd=Zd, m2=m2, Zd2=Zd2,
                row_img=row_img, row_txt=row_txt, row_L=row_L, r=r, c1=c1)


def shard_inputs(Ei, Et, En, logit_scale, cfg):
    """Per-core input dicts for the device kernel."""
    N, R, G, CH = cfg["N"], cfg["R"], cfg["G"], cfg["CH"]
    EiT = np.ascontiguousarray(Ei.T)
    EtT = np.ascontiguousarray(Et.T)
    ins = []
    for c in range(cfg["NCORES"]):
        sl = slice(R * c, R * (c + 1))
        mask = np.zeros((1, CH), np.float32)
        mask[0, c * G:(c + 1) * G] = 1.0
        ins.append({
            "eit_own": np.ascontiguousarray(EiT[:, sl]),
            "ett_own": np.ascontiguousarray(EtT[:, sl]),
            "eit_full": EiT,
            "ett_full": EtT,
            "ei_r": np.ascontiguousarray(Ei[sl]),
            "et_r": np.ascontiguousarray(Et[sl]),
            "en_r": np.ascontiguousarray(En[sl]),
            "lscale": np.array([[logit_scale]], np.float32),
            "own_mask": mask,
        })
    return ins


# ----------------------------------------------------------------------------
# device kernel
# ----------------------------------------------------------------------------

def build_kernel(tc, outs, ins, cfg, dbg=False):
    nc = tc.nc
    N, R, G, CH = cfg["N"], cfg["R"], cfg["G"], cfg["CH"]
    NCORES = cfg["NCORES"]
    nc_num = cfg["NC_NUM"]
    nsl = cfg["NSL"]
    NS = N // nsl
    rg = [list(range(NCORES))]
    pval = float(1.0 / N)
    qval = float(1.0 / (N + 1))
    lg = float(np.log(GAMMA))

    ctx = ExitStack()
    with ctx:
        big = ctx.enter_context(tc.tile_pool(name="big", bufs=1))
        sm = ctx.enter_context(tc.tile_pool(name="small", bufs=1))
        scr = ctx.enter_context(tc.tile_pool(name="scr", bufs=2))
        dram = ctx.enter_context(tc.tile_pool(name="dram", bufs=1, space="DRAM"))

        nc.gpsimd.load_library(library_config.attn)

        junk = sm.tile([1, 16], FT, tag="junk", name="junk")
        for ji, jk in enumerate(("eit_own", "ett_own", "eit_full", "ett_full",
                                 "ei_r", "et_r", "en_r", "lscale", "own_mask")):
            ap = ins[jk]
            idx = (slice(0, 1),) * len(ap.shape)
            src = ap[idx]
            if jk in ("eit_own", "ett_own", "eit_full", "ett_full"):
                src = src.bitcast(FT)
            nc.sync.dma_start(junk[0:1, ji:ji + 1], src)

        # ---------------- big SBUF arrays ----------------
        KA = big.tile([P, G, N], FR, tag="KA")
        KB = big.tile([P, G, N], FR, tag="KB")
        # tmp multi-use: ln scratch -> AR staging rows
        tmp = big.tile([P, N], FT, tag="tmp")
        # W (bf16 K*lnK) is allocated after the phase-1 pools close so the
        # stack allocator reuses their space; see below.
        W = None

        def ot(tag):   # own-vector tile [P, G]
            return sm.tile([P, G], FT, tag=tag, name=tag)

        def ft(tag):   # full-vector tile [P, CH]
            return sm.tile([P, CH], FT, tag=tag, name=tag)

        def st(tag):   # scalar broadcast tile [P, 1]
            return sm.tile([P, 1], FT, tag=tag, name=tag)

        svG = ot("svG")
        svCH = ft("svCH")

        def bcast_pack(dst, src1k, k):
            """[1,k] SBUF -> [128,k] SBUF via DRAM-bounce broadcast DMAs."""
            buf = dram.tile([k], FT, tag="bc%d" % k, name="bc%d" % k)
            nc.sync.dma_start(buf[:], src1k)
            for i in range(k):
                nc.sync.dma_start(dst[:, i:i + 1],
                                  buf[i:i + 1].to_broadcast((P, 1)))

        def tree_red(col, op=OP.add):
            """col [P, 1] -> scalar at col[0:1, 0:1] via partition-gather DMA."""
            row = scr.tile([1, P], FT, tag="tsrow", name="tsrow")
            nc.sync.dma_start(row[0:1, :], col[:, 0:1])
            nc.vector.tensor_reduce(out=col[0:1, 0:1], in_=row[0:1, :],
                                    axis=AX.X, op=op)

        # s = exp(logit_scale), broadcast early (needed for Z0 pass)
        ls11 = sm.tile([1, 1], FT, tag="ls11")
        nc.sync.dma_start(ls11[:], ins["lscale"][:])
        s11 = sm.tile([1, 1], FT, tag="s11")
        nc.scalar.activation(s11[:], ls11[:], AF.Exp)
        s_b = st("s_b")
        bcast_pack(s_b, s11[0:1, 0:1], 1)

        # own-shard column mask [P, CH]
        maskb = ft("maskb")
        nc.sync.dma_start(maskb[:], ins["own_mask"].to_broadcast((P, CH)))

        # ================= phase 1: dots =================
        d_o = ot("d_o"); sno_o = ot("sno_o"); cos_o = ot("cos_o")
        with tc.tile_pool(name="ph1dots", bufs=2) as ph1:
            for g in range(G):
                ei_g = ph1.tile([P, D], FT, tag="ei_g", name="ei_g")
                et_g = ph1.tile([P, D], FT, tag="et_g", name="et_g")
                en_g = ph1.tile([P, D], FT, tag="en_g", name="en_g")
                dsc = ph1.tile([P, D], FT, tag="dsc", name="dsc")
                rview_i = ins["ei_r"].rearrange("(g p) d -> g p d", p=P)
                rview_t = ins["et_r"].rearrange("(g p) d -> g p d", p=P)
                rview_n = ins["en_r"].rearrange("(g p) d -> g p d", p=P)
                nc.sync.dma_start(ei_g[:], rview_i[g])
                nc.sync.dma_start(et_g[:], rview_t[g])
                nc.sync.dma_start(en_g[:], rview_n[g])
                for (x1, x2, accum) in ((ei_g, et_g, d_o), (ei_g, en_g, sno_o),
                                        (et_g, en_g, cos_o)):
                    nc.vector.tensor_mul(dsc[:], x1[:], x2[:])
                    nc.vector.tensor_reduce(out=accum[:, g:g + 1], in_=dsc[:],
                                            axis=AX.X, op=OP.add)

        if cfg.get("STOP") == "dots":
            red = sm.tile([P, 1], FT, tag="eo_red", name="eo_red")
            nc.vector.tensor_reduce(out=red[:], in_=d_o[:], axis=AX.X, op=OP.add)
            tree_red(red)
            eo = sm.tile([1, 2], FT, tag="eo", name="eo")
            nc.vector.tensor_copy(eo[0:1, 0:1], red[0:1, 0:1])
            nc.vector.tensor_reduce(out=eo[0:1, 1:2], in_=junk[:], axis=AX.X,
                                    op=OP.add)
            nc.sync.dma_start(outs["out"][:], eo[:])
            return

        # ================= phase 1: sims matmuls =================
        # inputs are declared float32r in DRAM so no rounding pass is needed.
        with tc.tile_pool(name="mmps", bufs=2 * G, space="PSUM") as mmps, \
             tc.tile_pool(name="rhsp", bufs=2) as rhsp, \
             tc.tile_pool(name="lhsp", bufs=1) as lhsp:
            lhsAB = lhsp.tile([P, 2, KC, R], FR, tag="lhsAB", name="lhsAB")
            nc.sync.dma_start(lhsAB[:, 0],
                              ins["eit_own"].rearrange("(k p) r -> p k r", p=P))
            nc.sync.dma_start(lhsAB[:, 1],
                              ins["ett_own"].rearrange("(k p) r -> p k r", p=P))
            for li, (dst, rhs_dram) in enumerate(((KA, ins["ett_full"]),
                                                  (KB, ins["eit_full"]))):
                rview = rhs_dram.rearrange("(k p) j -> p k j", p=P)
                for n in range(N // PSL):
                    rhs_f = rhsp.tile([P, KC, PSL], FR, tag="rhs_f", name="rhs_f")
                    nc.sync.dma_start(rhs_f[:],
                                      rview[:, :, n * PSL:(n + 1) * PSL])
                    for g in range(G):
                        ps = mmps.tile([P, PSL], FT, tag="mm", name="mm")
                        for k in range(KC):
                            nc.tensor.matmul(
                                ps[:], lhsAB[:, li, k, g * P:(g + 1) * P],
                                rhs_f[:, k, :],
                                start=(k == 0), stop=(k == KC - 1))
                        nc.scalar.copy(dst[:, g, n * PSL:(n + 1) * PSL], ps[:])

        # W lives above the released phase-1 pools on the SBUF stack
        wpool = ctx.enter_context(tc.tile_pool(name="wpool", bufs=1))
        W = wpool.tile([P, G, N], BF, tag="W")

        # ---- scans on raw sims (KA/KB still hold sims here) ----
        Z0_o = ot("Z0_o")
        for g in range(G):
            # output values discarded into W's space (bf16); accum is fp32
            nc.scalar.activation(W[:, g, :], KA[:, g, :].bitcast(FT),
                                 AF.Exp, scale=s_b[:, 0:1],
                                 accum_out=Z0_o[:, g:g + 1])
        mnp = sm.tile([P, 4], FT, tag="mnp")
        nc.vector.tensor_reduce(out=mnp[:, 0:1], in_=KA[:, :, :].bitcast(FT),
                                axis=AX.XY, op=OP.min)
        nc.vector.tensor_reduce(out=mnp[:, 1:2], in_=sno_o[:], axis=AX.X,
                                op=OP.min)
        nc.vector.tensor_reduce(out=mnp[:, 0:1], in_=mnp[:, 0:2], axis=AX.X,
                                op=OP.min)
        tree_red(mnp[:, 0:1], OP.min)        # core-local min at [0,0]
        # sno max (for m2L)
        smx = sm.tile([P, 1], FT, tag="smx")
        nc.vector.tensor_reduce(out=smx[:], in_=sno_o[:], axis=AX.X, op=OP.max)
        tree_red(smx, OP.max)
        # hinge partial (loss_op)
        hu = ot("hu"); hw = ot("hw")
        nc.vector.tensor_scalar(out=hu[:], in0=cos_o[:], scalar1=0.2,
                                scalar2=0.0, op0=OP.add, op1=OP.max)
        nc.vector.tensor_scalar_mul(hw[:], cos_o[:], -1.0)
        nc.vector.tensor_scalar(out=hw[:], in0=hw[:], scalar1=-0.7,
                                scalar2=0.0, op0=OP.add, op1=OP.max)
        nc.vector.tensor_add(hu[:], hu[:], hw[:])
        hcol = sm.tile([P, 1], FT, tag="hcol")
        nc.vector.tensor_reduce(out=hcol[:], in_=hu[:], axis=AX.X, op=OP.add)
        tree_red(hcol)

        # rowmax(KA raw) / colmax(KB raw) -> m_o, m2_o (pre-transform!)
        m_o = ot("m_o"); m2_o = ot("m2_o")
        nc.vector.tensor_reduce(out=m_o[:], in_=KA[:, :, :].bitcast(FT),
                                axis=AX.X, op=OP.max)
        nc.vector.tensor_max(m_o[:], m_o[:], sno_o[:])
        nc.vector.tensor_scalar(out=m_o[:], in0=m_o[:], scalar1=s_b[:, 0:1],
                                scalar2=None, op0=OP.mult)
        nc.vector.tensor_reduce(out=m2_o[:], in_=KB[:, :, :].bitcast(FT),
                                axis=AX.X, op=OP.max)
        nc.vector.tensor_scalar(out=m2_o[:], in0=m2_o[:], scalar1=s_b[:, 0:1],
                                scalar2=None, op0=OP.mult)

        # tau = s*d - ln(Z0)
        tau_o = ot("tau_o")
        nc.scalar.activation(tau_o[:], Z0_o[:], AF.Ln)
        nc.vector.tensor_scalar(out=svG[:], in0=d_o[:], scalar1=s_b[:, 0:1],
                                scalar2=None, op0=OP.mult)
        nc.vector.tensor_sub(tau_o[:], svG[:], tau_o[:])

        if cfg.get("STOP") == "mm":
            red = sm.tile([P, 1], FT, tag="eo_red", name="eo_red")
            nc.vector.tensor_reduce(out=red[:], in_=tau_o[:], axis=AX.X,
                                    op=OP.add)
            tree_red(red)
            eo = sm.tile([1, 2], FT, tag="eo", name="eo")
            nc.vector.tensor_copy(eo[0:1, 0:1], red[0:1, 0:1])
            nc.vector.tensor_copy(eo[0:1, 1:2], mnp[0:1, 0:1])
            nc.sync.dma_start(outs["out"][:], eo[:])
            return

        # ---------------- AG#0: tau, sno, d, pads ----------------
        SLAB0 = 3 * R + 8
        ag0_in = dram.tile([SLAB0], FT, tag="ag0i")
        ag0_out = dram.tile([NCORES, SLAB0], FT, tag="ag0o")
        nc.sync.dma_start(ag0_in[0:R].rearrange("(g p) -> p g", p=P), tau_o[:])
        nc.sync.dma_start(ag0_in[R:2 * R].rearrange("(g p) -> p g", p=P), sno_o[:])
        nc.sync.dma_start(ag0_in[2 * R:3 * R].rearrange("(g p) -> p g", p=P), d_o[:])
        pad8 = sm.tile([1, 8], FT, tag="pad8", name="pad8")
        nc.vector.memset(pad8[:], 0.0)
        nc.vector.tensor_copy(pad8[0:1, 0:1], mnp[0:1, 0:1])
        nc.vector.tensor_copy(pad8[0:1, 1:2], hcol[0:1, 0:1])
        nc.vector.tensor_copy(pad8[0:1, 2:3], smx[0:1, 0:1])
        nc.sync.dma_start(ag0_in[3 * R:3 * R + 8], pad8[:])
        nc.gpsimd.collective_compute(
            "AllGather", OP.bypass, ins=[ag0_in.opt()], outs=[ag0_out.opt()],
            replica_groups=rg)

        tau_f = ft("tau_f"); sno_f = ft("sno_f"); d_f = ft("d_f")
        for c in range(NCORES):
            nc.sync.dma_start(
                tau_f[:, c * G:(c + 1) * G],
                ag0_out[c, 0:R].rearrange("(g p) -> p g", p=P))
            nc.sync.dma_start(
                sno_f[:, c * G:(c + 1) * G],
                ag0_out[c, R:2 * R].rearrange("(g p) -> p g", p=P))
            nc.sync.dma_start(
                d_f[:, c * G:(c + 1) * G],
                ag0_out[c, 2 * R:3 * R].rearrange("(g p) -> p g", p=P))
        pads8 = sm.tile([1, 3 * NCORES], FT, tag="pads8")
        for k in range(3):
            nc.sync.dma_start(
                pads8[0:1, k * NCORES:(k + 1) * NCORES],
                ag0_out[:, 3 * R + k:3 * R + k + 1].rearrange("c x -> x c"))
        mn11 = sm.tile([1, 1], FT, tag="mn11")
        nc.vector.tensor_reduce(out=mn11[:], in_=pads8[0:1, 0:NCORES],
                                axis=AX.X, op=OP.min)
        hng11 = sm.tile([1, 1], FT, tag="hng11")
        nc.vector.tensor_reduce(out=hng11[:],
                                in_=pads8[0:1, NCORES:2 * NCORES],
                                axis=AX.X, op=OP.add)
        m2L11 = sm.tile([1, 1], FT, tag="m2L11")
        nc.vector.tensor_reduce(out=m2L11[:],
                                in_=pads8[0:1, 2 * NCORES:3 * NCORES],
                                axis=AX.X, op=OP.max)
        nc.vector.tensor_mul(m2L11[:], m2L11[:], s11[:])

        # alpha = REG*(1-mn); pack [alpha, 1/alpha, -1/alpha, s*alpha]
        sc4 = sm.tile([1, 4], FT, tag="sc4")
        nc.scalar.activation(sc4[0:1, 0:1], mn11[:], AF.Identity,
                             bias=1.0, scale=-1.0)
        nc.scalar.mul(sc4[0:1, 0:1], sc4[0:1, 0:1], REG)
        nc.vector.reciprocal(sc4[0:1, 1:2], sc4[0:1, 0:1])
        nc.scalar.mul(sc4[0:1, 2:3], sc4[0:1, 1:2], -1.0)
        nc.vector.tensor_mul(sc4[0:1, 3:4], s11[:], sc4[0:1, 0:1])
        scb = sm.tile([P, 4], FT, tag="scb")
        bcast_pack(scb, sc4[0:1, :], 4)
        al_b = scb[:, 0:1]; ial_b = scb[:, 1:2]
        nial_b = scb[:, 2:3]; sal_b = scb[:, 3:4]

        # ---------------- kth smallest -> nc masks ----------------
        ntau_f = ft("ntau_f")
        nc.vector.tensor_scalar_mul(ntau_f[:], tau_f[:], -1.0)
        kth = sm.tile([1, 2], FT, tag="kth")
        qk = 1.0 - (nc_num - 0.5) / (N - 1)
        nc.gpsimd.kth_largest(kth[:], ntau_f[:], n_per_lane=CH, k=nc_num + 1,
                              quantile=qk)
        nthr_b = st("nthr_b")
        bcast_pack(nthr_b, kth[0:1, 0:1], 1)
        nc_f = ft("nc_f")
        nc.vector.tensor_scalar(out=nc_f[:], in0=ntau_f[:],
                                scalar1=nthr_b[:, 0:1], scalar2=None,
                                op0=OP.is_gt)
        nc_o = ot("nc_o")
        nc.vector.tensor_scalar_mul(svG[:], tau_o[:], -1.0)
        nc.vector.tensor_scalar(out=nc_o[:], in0=svG[:],
                                scalar1=nthr_b[:, 0:1], scalar2=None,
                                op0=OP.is_gt)

        # ---------------- transform sims -> K (in place) ----------------
        # KA transform accumulates r = rowsum(K) per own row.
        rA_o = ot("rA_o")
        for g in range(G):
            nc.scalar.activation(KA[:, g, :], KA[:, g, :].bitcast(FT), AF.Exp,
                                 scale=ial_b, bias=nial_b,
                                 accum_out=rA_o[:, g:g + 1])
        for g in range(G):
            nc.scalar.activation(KB[:, g, :], KB[:, g, :].bitcast(FT), AF.Exp,
                                 scale=ial_b, bias=nial_b)

        # Kd/Klc (full + own), nKd/nKlc
        Kd_f = ft("Kd_f"); Klc_f = ft("Klc_f")
        nc.scalar.activation(Kd_f[:], d_f[:], AF.Exp, scale=ial_b, bias=nial_b)
        nc.scalar.activation(Klc_f[:], sno_f[:], AF.Exp, scale=ial_b,
                             bias=nial_b)
        nKd_f = ft("nKd_f"); nKlc_f = ft("nKlc_f")
        nc.vector.tensor_mul(nKd_f[:], nc_f[:], Kd_f[:])
        nc.vector.tensor_mul(nKlc_f[:], nc_f[:], Klc_f[:])
        Kd_o = ot("Kd_o"); Klc_o = ot("Klc_o")
        nc.scalar.activation(Kd_o[:], d_o[:], AF.Exp, scale=ial_b, bias=nial_b)
        nc.scalar.activation(Klc_o[:], sno_o[:], AF.Exp, scale=ial_b,
                             bias=nial_b)
        nKd_o = ot("nKd_o"); nKlc_o = ot("nKlc_o")
        nc.vector.tensor_mul(nKd_o[:], nc_o[:], Kd_o[:])
        nc.vector.tensor_mul(nKlc_o[:], nc_o[:], Klc_o[:])

        def early_out(t_):
            red = sm.tile([P, 1], FT, tag="eo_red", name="eo_red")
            nc.vector.tensor_reduce(out=red[:], in_=t_[:], axis=AX.X, op=OP.add)
            tree_red(red)
            eo = sm.tile([1, 2], FT, tag="eo", name="eo")
            nc.vector.tensor_copy(eo[0:1, 0:1], red[0:1, 0:1])
            nc.vector.tensor_reduce(out=eo[0:1, 1:2], in_=junk[:], axis=AX.X,
                                    op=OP.add)
            nc.sync.dma_start(outs["out"][:], eo[:])

        if cfg.get("STOP") == "phase1":
            early_out(nc_o)
            return

        # ================= sinkhorn T=1, collapsed =================
        # a1 = p / (r - nKd + nKlc)   (own rows; r from transform accum)
        a1_o = ot("a1_o")
        nc.vector.tensor_sub(svG[:], rA_o[:], nKd_o[:])
        nc.vector.tensor_add(svG[:], svG[:], nKlc_o[:])
        nc.vector.reciprocal(a1_o[:], svG[:])
        nc.vector.tensor_scalar_mul(a1_o[:], a1_o[:], pval)

        lhs2 = sm.tile([P, 2 * G], FR, tag="lhs2")
        lhsW = sm.tile([P, G], BF, tag="lhsW")
        mvps = ctx.enter_context(tc.tile_pool(name="mvps", bufs=1, space="PSUM"))

        def matvec(KM, M, psname):
            ps = mvps.tile([3, N], FT, tag="mv", name=psname)
            lview = lhs2[:, 0:M * G].rearrange("p (v g) -> p g v", g=G)
            for n in range(NS):
                for g in range(G):
                    nc.tensor.matmul(
                        ps[0:M, n * nsl:(n + 1) * nsl],
                        lview[:, g, :],
                        KM[:, g, n * nsl:(n + 1) * nsl],
                        start=(g == 0), stop=(g == G - 1))
            return ps

        # MV_A1: c1_partial = K^T @ a1 over own rows
        nc.vector.tensor_copy(lhs2[:, 0:G], a1_o[:])
        ps1 = matvec(KA, 1, "ps1")

        # zero-padded own contribution helper: out_f = spread(own) * maskb
        def zp(out_f, own):
            for c in range(NCORES):
                nc.vector.tensor_copy(out_f[:, c * G:(c + 1) * G], own[:])
            nc.vector.tensor_mul(out_f[:], out_f[:], maskb[:])

        # AR_A1 payload: [c1_raw partials; r zero-padded]
        ar1_in = dram.tile([2, N], FT, tag="ar1i")
        ar1_out = dram.tile([2, N], FT, tag="ar1o")
        nc.scalar.copy(tmp[0:1, :], ps1[0:1, :])
        nc.sync.dma_start(ar1_in[0, :], tmp[0:1, :])
        rzp = ft("rzp")
        zp(rzp, rA_o)
        nc.sync.dma_start(ar1_in[1, :].rearrange("(k p) -> p k", p=P), rzp[:])
        nc.gpsimd.collective_compute(
            "AllReduce", OP.add, ins=[ar1_in.opt()], outs=[ar1_out.opt()],
            replica_groups=rg)

        # ---- W_B = KB*lnKB (bf16) + Zd2/m2 prep: hidden under AR_A1 ----
        # Per g: Ln -> tmp; per half: DVE mul (W chunk), then in-place exp on
        # tmp with accum (half-splitting pipelines the Act/DVE ping-pong).
        Zd2_o = ot("Zd2_o")
        smb2_o = ot("smb2_o")   # s - m2
        nc.vector.tensor_scalar_mul(smb2_o[:], m2_o[:], -1.0)
        nc.vector.tensor_scalar(out=smb2_o[:], in0=smb2_o[:],
                                scalar1=s_b[:, 0:1], scalar2=None, op0=OP.add)
        H = N // 2
        za = sm.tile([P, 2 * G], FT, tag="za")

        def w_zd_prep(KM, smb, zd_out):
            for g in range(G):
                nc.scalar.activation(tmp[:, :], KM[:, g, :].bitcast(FT), AF.Ln)
                for h in range(2):
                    sl = slice(h * H, (h + 1) * H)
                    nc.vector.tensor_mul(W[:, g, sl], KM[:, g, sl].bitcast(FT),
                                         tmp[:, sl])
                    nc.scalar.activation(tmp[:, sl], tmp[:, sl], AF.Exp,
                                         scale=sal_b, bias=smb[:, g:g + 1],
                                         accum_out=za[:, h * G + g:h * G + g + 1])
            nc.vector.tensor_add(zd_out[:], za[:, 0:G], za[:, G:2 * G])

        w_zd_prep(KB, smb2_o, Zd2_o)

        # ---- post-AR_A1: b1 full everywhere ----
        c1r_f = ft("c1r_f"); r_f = ft("r_f")
        nc.sync.dma_start(c1r_f[:], ar1_out[0, :].rearrange("(k p) -> p k", p=P))
        nc.sync.dma_start(r_f[:], ar1_out[1, :].rearrange("(k p) -> p k", p=P))
        a1_f = ft("a1_f")
        nc.vector.tensor_sub(svCH[:], r_f[:], nKd_f[:])
        nc.vector.tensor_add(svCH[:], svCH[:], nKlc_f[:])
        nc.vector.reciprocal(a1_f[:], svCH[:])
        nc.vector.tensor_scalar_mul(a1_f[:], a1_f[:], pval)
        b1_f = ft("b1_f")
        nc.vector.tensor_mul(svCH[:], nKd_f[:], a1_f[:])
        nc.vector.tensor_sub(svCH[:], c1r_f[:], svCH[:])
        nc.vector.reciprocal(b1_f[:], svCH[:])
        nc.vector.tensor_scalar_mul(b1_f[:], b1_f[:], qval)
        # b1L = a2L = q / dot(nKlc, a1)
        bl11 = sm.tile([1, 2], FT, tag="bl11")
        col = scr.tile([P, 1], FT, tag="colsum", name="colsum")
        nc.vector.tensor_mul(svCH[:], nKlc_f[:], a1_f[:])
        nc.vector.tensor_reduce(out=col[:], in_=svCH[:], axis=AX.X, op=OP.add)
        tree_red(col)
        nc.vector.reciprocal(bl11[0:1, 0:1], col[0:1, 0:1])
        nc.vector.tensor_scalar_mul(bl11[0:1, 0:1], bl11[0:1, 0:1], qval)
        nc.scalar.activation(bl11[0:1, 1:2], bl11[0:1, 0:1], AF.Ln)
        blb = sm.tile([P, 2], FT, tag="blb")
        bcast_pack(blb, bl11[0:1, :], 2)
        b1L = blb[:, 0:1]; lnb1L = blb[:, 1:2]

        if cfg.get("STOP") == "sink":
            early_out(b1_f)
            return

        # own-col slice of a full vector: mask, then sum the 8 blocks
        def extract(own_out, full):
            nc.vector.tensor_mul(svCH[:], full[:], maskb[:])
            nc.vector.tensor_copy(own_out[:], svCH[:, 0:G])
            for c in range(1, NCORES):
                nc.vector.tensor_add(own_out[:], own_out[:],
                                     svCH[:, c * G:(c + 1) * G])

        # ================= final B* =================
        b1_oc = ot("b1_oc")
        extract(b1_oc, b1_f)
        blb1_oc = ot("blb1_oc")
        nc.scalar.activation(blb1_oc[:], b1_oc[:], AF.Ln)
        nc.vector.tensor_mul(blb1_oc[:], blb1_oc[:], b1_oc[:])
        nc.vector.tensor_copy(lhs2[:, 0:G], b1_oc[:])
        nc.vector.tensor_copy(lhs2[:, G:2 * G], blb1_oc[:])
        nc.vector.tensor_copy(lhsW[:], b1_oc[:])

        # AR_F1 payload: [M1, T3, T2, m2_zp, Zd2_zp]
        arf1_in = dram.tile([5, N], FT, tag="arf1i")
        arf1_out = dram.tile([5, N], FT, tag="arf1o")
        ps3 = matvec(KB, 2, "ps3")          # rows: M1, T3
        nc.scalar.copy(tmp[0:2, :], ps3[0:2, :])
        nc.sync.dma_start(arf1_in[0:2, :], tmp[0:2, :])
        psT2 = mvps.tile([3, N], FT, tag="mv", name="psT2")
        for n in range(NS):
            for g in range(G):
                nc.tensor.matmul(
                    psT2[0:1, n * nsl:(n + 1) * nsl],
                    lhsW[:, g:g + 1],
                    W[:, g, n * nsl:(n + 1) * nsl],
                    start=(g == 0), stop=(g == G - 1))
        nc.scalar.copy(tmp[0:1, :], psT2[0:1, :])
        nc.sync.dma_start(arf1_in[2, :], tmp[0:1, :])
        m2zp = ft("m2zp"); zd2zp = ft("zd2zp")
        zp(m2zp, m2_o)
        zp(zd2zp, Zd2_o)
        nc.sync.dma_start(arf1_in[3, :].rearrange("(k p) -> p k", p=P), m2zp[:])
        nc.sync.dma_start(arf1_in[4, :].rearrange("(k p) -> p k", p=P), zd2zp[:])
        nc.gpsimd.collective_compute(
            "AllReduce", OP.add, ins=[arf1_in.opt()], outs=[arf1_out.opt()],
            replica_groups=rg)

        # ---- W_A = KA*lnKA + Zd/m prep: hidden under AR_F1 ----
        Zd_o = ot("Zd_o")
        smb_o = ot("smb_o")     # s - m
        nc.vector.tensor_scalar_mul(smb_o[:], m_o[:], -1.0)
        nc.vector.tensor_scalar(out=smb_o[:], in0=smb_o[:],
                                scalar1=s_b[:, 0:1], scalar2=None, op0=OP.add)
        w_zd_prep(KA, smb_o, Zd_o)
        mzp = ft("mzp"); zdzp = ft("zdzp")
        zp(mzp, m_o)
        zp(zdzp, Zd_o)

        # ---- post-AR_F1 ----
        M1_f = ft("M1_f"); T3_f = ft("T3_f"); T2_f = ft("T2_f")
        m2_f = ft("m2_f"); Zd2_f = ft("Zd2_f")
        for i, f in enumerate((M1_f, T3_f, T2_f, m2_f, Zd2_f)):
            nc.sync.dma_start(f[:], arf1_out[i, :].rearrange("(k p) -> p k", p=P))
        # R1 = M1 - nKd*b1 + nKlc*b1L ; b2 = p/R1
        R1_f = ft("R1_f")
        nc.vector.tensor_mul(svCH[:], nKd_f[:], b1_f[:])
        nc.vector.tensor_sub(R1_f[:], M1_f[:], svCH[:])
        nc.vector.tensor_scalar(out=svCH[:], in0=nKlc_f[:], scalar1=b1L,
                                scalar2=None, op0=OP.mult)
        nc.vector.tensor_add(R1_f[:], R1_f[:], svCH[:])
        b2_f = ft("b2_f")
        nc.vector.reciprocal(b2_f[:], R1_f[:])
        nc.vector.tensor_scalar_mul(b2_f[:], b2_f[:], pval)

        # ================= final A* =================
        b2_or = ot("b2_or")
        extract(b2_or, b2_f)
        blb2_or = ot("blb2_or")
        nc.scalar.activation(blb2_or[:], b2_or[:], AF.Ln)
        nc.vector.tensor_mul(blb2_or[:], blb2_or[:], b2_or[:])
        nc.vector.tensor_copy(lhs2[:, 0:G], b2_or[:])
        nc.vector.tensor_copy(lhs2[:, G:2 * G], blb2_or[:])
        nc.vector.tensor_copy(lhsW[:], b2_or[:])

        # AR_F2 payload: [M1p, T3p, T2p, m_zp, Zd_zp]
        arf2_in = dram.tile([5, N], FT, tag="arf2i")
        arf2_out = dram.tile([5, N], FT, tag="arf2o")
        ps2 = matvec(KA, 2, "ps2")          # rows: M1p, T3p
        nc.scalar.copy(tmp[0:2, :], ps2[0:2, :])
        nc.sync.dma_start(arf2_in[0:2, :], tmp[0:2, :])
        psT2b = mvps.tile([3, N], FT, tag="mv", name="psT2b")
        for n in range(NS):
            for g in range(G):
                nc.tensor.matmul(
                    psT2b[0:1, n * nsl:(n + 1) * nsl],
                    lhsW[:, g:g + 1],
                    W[:, g, n * nsl:(n + 1) * nsl],
                    start=(g == 0), stop=(g == G - 1))
        nc.scalar.copy(tmp[0:1, :], psT2b[0:1, :])
        nc.sync.dma_start(arf2_in[2, :], tmp[0:1, :])
        nc.sync.dma_start(arf2_in[3, :].rearrange("(k p) -> p k", p=P), mzp[:])
        nc.sync.dma_start(arf2_in[4, :].rearrange("(k p) -> p k", p=P), zdzp[:])
        nc.gpsimd.collective_compute(
            "AllReduce", OP.add, ins=[arf2_in.opt()], outs=[arf2_out.opt()],
            replica_groups=rg)

        # ---- row_L prep + row_img partials: hidden under AR_F2 ----
        lnKd_f = ft("lnKd_f"); lnKlc_f = ft("lnKlc_f")
        nc.vector.tensor_scalar(out=lnKd_f[:], in0=d_f[:], scalar1=-1.0,
                                scalar2=None, op0=OP.add)
        nc.vector.tensor_scalar(out=lnKd_f[:], in0=lnKd_f[:], scalar1=ial_b,
                                scalar2=None, op0=OP.mult)
        nc.vector.tensor_scalar(out=lnKlc_f[:], in0=sno_f[:], scalar1=-1.0,
                                scalar2=None, op0=OP.add)
        nc.vector.tensor_scalar(out=lnKlc_f[:], in0=lnKlc_f[:], scalar1=ial_b,
                                scalar2=None, op0=OP.mult)
        lnb1_f = ft("lnb1_f")
        nc.scalar.activation(lnb1_f[:], b1_f[:], AF.Ln)
        lnb2_f = ft("lnb2_f")
        nc.scalar.activation(lnb2_f[:], b2_f[:], AF.Ln)
        rR1 = ft("rR1")
        nc.vector.tensor_scalar_mul(rR1[:], b2_f[:], float(N))  # 1/R1 = b2*N

        acc = ft("acc"); u = ft("u"); w = ft("w")
        # KbS = M1 + alpha*T2 - nKd*b1*d + nKlc*b1L*sno
        nc.vector.tensor_scalar(out=acc[:], in0=T2_f[:], scalar1=al_b,
                                scalar2=None, op0=OP.mult)
        nc.vector.tensor_add(acc[:], acc[:], M1_f[:])
        nc.vector.tensor_mul(u[:], nKd_f[:], b1_f[:])
        nc.vector.tensor_mul(u[:], u[:], d_f[:])
        nc.vector.tensor_sub(acc[:], acc[:], u[:])
        nc.vector.tensor_scalar(out=u[:], in0=nKlc_f[:], scalar1=b1L,
                                scalar2=None, op0=OP.mult)
        nc.vector.tensor_mul(u[:], u[:], sno_f[:])
        nc.vector.tensor_add(acc[:], acc[:], u[:])            # KbS
        nc.vector.tensor_mul(acc[:], acc[:], rR1[:])
        nc.vector.tensor_scalar_mul(acc[:], acc[:], GAMMA)
        nc.vector.tensor_mul(u[:], nc_f[:], sno_f[:])
        nc.vector.tensor_mul(w[:], nc_f[:], d_f[:])
        nc.vector.tensor_sub(w[:], d_f[:], w[:])
        nc.vector.tensor_add(u[:], u[:], w[:])
        nc.vector.tensor_scalar_mul(u[:], u[:], 1.0 - GAMMA)
        nc.vector.tensor_add(acc[:], acc[:], u[:])            # TS
        nc.vector.tensor_scalar(out=acc[:], in0=acc[:], scalar1=s_b[:, 0:1],
                                scalar2=None, op0=OP.mult)
        nc.vector.tensor_scalar_mul(acc[:], acc[:], -1.0)     # -s*TS
        # entropy: wlnw = T2 + T3 - nKd*b1*(lnKd+lnb1) + nKlc*b1L*(lnKlc+lnb1L)
        ent = ft("ent")
        nc.vector.tensor_add(ent[:], T2_f[:], T3_f[:])
        nc.vector.tensor_add(u[:], lnKd_f[:], lnb1_f[:])
        nc.vector.tensor_mul(u[:], u[:], nKd_f[:])
        nc.vector.tensor_mul(u[:], u[:], b1_f[:])
        nc.vector.tensor_sub(ent[:], ent[:], u[:])
        nc.vector.tensor_scalar(out=u[:], in0=lnKlc_f[:], scalar1=lnb1L,
                                scalar2=None, op0=OP.add)
        nc.vector.tensor_mul(u[:], u[:], nKlc_f[:])
        nc.vector.tensor_scalar(out=u[:], in0=u[:], scalar1=b1L,
                                scalar2=None, op0=OP.mult)
        nc.vector.tensor_add(ent[:], ent[:], u[:])            # wlnw
        nc.vector.tensor_mul(ent[:], ent[:], rR1[:])
        nc.scalar.activation(u[:], R1_f[:], AF.Ln)
        nc.vector.tensor_sub(ent[:], ent[:], u[:])            # sum P lnP
        psp = ft("psp")
        nc.vector.tensor_scalar(out=psp[:], in0=nKlc_f[:], scalar1=b1L,
                                scalar2=None, op0=OP.mult)
        nc.vector.tensor_mul(u[:], nc_f[:], Kd_f[:])
        nc.vector.tensor_sub(u[:], Kd_f[:], u[:])
        nc.vector.tensor_mul(u[:], u[:], b1_f[:])
        nc.vector.tensor_add(psp[:], psp[:], u[:])
        nc.vector.tensor_mul(psp[:], psp[:], rR1[:])
        lnpsp = ft("lnpsp")
        nc.scalar.activation(lnpsp[:], psp[:], AF.Ln)
        nc.vector.tensor_mul(u[:], psp[:], lnpsp[:])
        nc.vector.tensor_sub(ent[:], ent[:], u[:])
        nc.vector.tensor_scalar_mul(ent[:], ent[:], GAMMA)
        nc.vector.tensor_scalar_mul(u[:], psp[:], -GAMMA * lg)
        nc.vector.tensor_add(ent[:], ent[:], u[:])
        nc.vector.tensor_scalar(out=ent[:], in0=ent[:], scalar1=GAMMA * lg,
                                scalar2=None, op0=OP.add)
        tsp = ft("tsp")
        nc.vector.tensor_scalar_mul(tsp[:], psp[:], GAMMA)
        nc.vector.tensor_scalar(out=tsp[:], in0=tsp[:], scalar1=1.0 - GAMMA,
                                scalar2=None, op0=OP.add)
        nc.scalar.activation(u[:], tsp[:], AF.Ln)
        nc.vector.tensor_mul(u[:], u[:], tsp[:])
        nc.vector.tensor_add(ent[:], ent[:], u[:])
        nc.vector.tensor_add(acc[:], acc[:], ent[:])   # row_img sans m + lnZ

        # row_L pieces not needing AR_F2: Z2L from sno_f, sum(nc*sno)
        nm2L = st("nm2L")
        r11 = sm.tile([1, 1], FT, tag="r11")
        nc.scalar.mul(r11[:], m2L11[:], -1.0)
        bcast_pack(nm2L, r11[0:1, 0:1], 1)
        fCH2 = ft("fCH2")
        nc.vector.tensor_scalar(out=fCH2[:], in0=sno_f[:], scalar1=s_b[:, 0:1],
                                scalar2=None, op0=OP.mult)
        nc.vector.tensor_scalar(out=fCH2[:], in0=fCH2[:], scalar1=nm2L[:, 0:1],
                                scalar2=None, op0=OP.add)
        nc.scalar.activation(fCH2[:], fCH2[:], AF.Exp)
        Z2L11 = sm.tile([1, 1], FT, tag="Z2L11")
        nc.vector.tensor_reduce(out=col[:], in_=fCH2[:], axis=AX.X, op=OP.add)
        tree_red(col)
        nc.vector.tensor_copy(Z2L11[:], col[0:1, 0:1])
        ncsno11 = sm.tile([1, 1], FT, tag="ncsno11")
        nc.vector.tensor_mul(fCH2[:], nc_f[:], sno_f[:])
        nc.vector.tensor_reduce(out=col[:], in_=fCH2[:], axis=AX.X, op=OP.add)
        tree_red(col)
        nc.vector.tensor_copy(ncsno11[:], col[0:1, 0:1])
        # R2L = dot(nKlc, b2)
        R2L11 = sm.tile([1, 1], FT, tag="R2L11")
        nc.vector.tensor_mul(fCH2[:], nKlc_f[:], b2_f[:])
        nc.vector.tensor_reduce(out=col[:], in_=fCH2[:], axis=AX.X, op=OP.add)
        tree_red(col)
        nc.vector.tensor_copy(R2L11[:], col[0:1, 0:1])
        rR2L = st("rR2L")
        nc.vector.reciprocal(r11[:], R2L11[:])
        bcast_pack(rR2L, r11[0:1, 0:1], 1)
        # P2L = nKlc*b2/R2L ; TSLa = sum(P2L*sno) ; EntL = sum tL ln(tL+(1-nc))
        fCH = ft("fCH")
        nc.vector.tensor_mul(fCH[:], nKlc_f[:], b2_f[:])
        nc.vector.tensor_scalar(out=fCH[:], in0=fCH[:], scalar1=rR2L[:, 0:1],
                                scalar2=None, op0=OP.mult)
        TSLa11 = sm.tile([1, 1], FT, tag="TSLa11")
        nc.vector.tensor_mul(fCH2[:], fCH[:], sno_f[:])
        nc.vector.tensor_reduce(out=col[:], in_=fCH2[:], axis=AX.X, op=OP.add)
        tree_red(col)
        nc.vector.tensor_copy(TSLa11[:], col[0:1, 0:1])
        nc.vector.tensor_scalar_mul(fCH[:], fCH[:], GAMMA)
        nc.vector.tensor_scalar_mul(fCH2[:], nc_f[:], 1.0 - GAMMA)
        nc.vector.tensor_add(fCH[:], fCH[:], fCH2[:])         # tL
        nc.vector.tensor_scalar_mul(fCH2[:], nc_f[:], -1.0)
        nc.vector.tensor_scalar(out=fCH2[:], in0=fCH2[:], scalar1=1.0,
                                scalar2=None, op0=OP.add)
        nc.vector.tensor_add(fCH2[:], fCH2[:], fCH[:])        # tL + (1-nc)
        nc.scalar.activation(fCH2[:], fCH2[:], AF.Ln)
        nc.vector.tensor_mul(fCH2[:], fCH2[:], fCH[:])
        entL11 = sm.tile([1, 1], FT, tag="entL11")
        nc.vector.tensor_reduce(out=col[:], in_=fCH2[:], axis=AX.X, op=OP.add)
        tree_red(col)
        nc.vector.tensor_copy(entL11[:], col[0:1, 0:1])

        # ---- post-AR_F2 ----
        M1p_f = ft("M1p_f"); T3p_f = ft("T3p_f"); T2p_f = ft("T2p_f")
        m_f = ft("m_f"); Zd_f = ft("Zd_f")
        for i, f in enumerate((M1p_f, T3p_f, T2p_f, m_f, Zd_f)):
            nc.sync.dma_start(f[:], arf2_out[i, :].rearrange("(k p) -> p k", p=P))

        # finish row_img: + m + ln(Zd + exp(s*sno - m))
        Zf = ft("Zf")
        nc.vector.tensor_scalar(out=Zf[:], in0=sno_f[:], scalar1=s_b[:, 0:1],
                                scalar2=None, op0=OP.mult)
        nc.vector.tensor_sub(Zf[:], Zf[:], m_f[:])
        nc.scalar.activation(Zf[:], Zf[:], AF.Exp)
        nc.vector.tensor_add(Zf[:], Zf[:], Zd_f[:])
        nc.scalar.activation(u[:], Zf[:], AF.Ln)
        nc.vector.tensor_add(acc[:], acc[:], u[:])
        nc.vector.tensor_add(acc[:], acc[:], m_f[:])          # row_img
        rimg = sm.tile([P, 1], FT, tag="rimg")
        nc.vector.tensor_reduce(out=rimg[:], in_=acc[:], axis=AX.X, op=OP.add)
        tree_red(rimg)

        # ---------------- row_txt assembly ----------------
        R2_f = ft("R2_f")
        nc.vector.tensor_mul(svCH[:], nKd_f[:], b2_f[:])
        nc.vector.tensor_sub(R2_f[:], M1p_f[:], svCH[:])
        rR2 = ft("rR2")
        nc.vector.reciprocal(rR2[:], R2_f[:])
        acc2 = ft("acc2")
        nc.vector.tensor_scalar(out=acc2[:], in0=T2p_f[:], scalar1=al_b,
                                scalar2=None, op0=OP.mult)
        nc.vector.tensor_add(acc2[:], acc2[:], M1p_f[:])
        nc.vector.tensor_mul(u[:], nKd_f[:], b2_f[:])
        nc.vector.tensor_mul(u[:], u[:], d_f[:])
        nc.vector.tensor_sub(acc2[:], acc2[:], u[:])          # KbS2
        nc.vector.tensor_mul(acc2[:], acc2[:], rR2[:])
        nc.vector.tensor_scalar_mul(acc2[:], acc2[:], GAMMA)
        nc.vector.tensor_mul(w[:], nc_f[:], d_f[:])
        nc.vector.tensor_sub(w[:], d_f[:], w[:])
        nc.vector.tensor_scalar_mul(w[:], w[:], 1.0 - GAMMA)
        nc.vector.tensor_add(acc2[:], acc2[:], w[:])          # TS2
        nc.vector.tensor_scalar(out=acc2[:], in0=acc2[:], scalar1=s_b[:, 0:1],
                                scalar2=None, op0=OP.mult)
        nc.vector.tensor_scalar_mul(acc2[:], acc2[:], -1.0)
        nc.scalar.activation(u[:], Zd2_f[:], AF.Ln)
        nc.vector.tensor_add(u[:], u[:], m2_f[:])
        st2 = ft("st2")
        nc.vector.tensor_scalar_mul(st2[:], nc_f[:], -(1.0 - GAMMA))
        nc.vector.tensor_scalar(out=st2[:], in0=st2[:], scalar1=1.0,
                                scalar2=None, op0=OP.add)
        nc.vector.tensor_mul(u[:], u[:], st2[:])
        nc.vector.tensor_add(acc2[:], acc2[:], u[:])
        # entropy2: wlnw2 = T2p + T3p - nKd*b2*(lnKd + lnb2)
        nc.vector.tensor_add(ent[:], T2p_f[:], T3p_f[:])
        nc.vector.tensor_add(u[:], lnKd_f[:], lnb2_f[:])
        nc.vector.tensor_mul(u[:], u[:], nKd_f[:])
        nc.vector.tensor_mul(u[:], u[:], b2_f[:])
        nc.vector.tensor_sub(ent[:], ent[:], u[:])
        nc.vector.tensor_mul(ent[:], ent[:], rR2[:])
        nc.scalar.activation(u[:], R2_f[:], AF.Ln)
        nc.vector.tensor_sub(ent[:], ent[:], u[:])            # sum P lnP (2)
        nc.vector.tensor_mul(psp[:], nc_f[:], Kd_f[:])
        nc.vector.tensor_sub(psp[:], Kd_f[:], psp[:])
        nc.vector.tensor_mul(psp[:], psp[:], b2_f[:])
        nc.vector.tensor_mul(psp[:], psp[:], rR2[:])          # Psp2
        nc.vector.tensor_add(u[:], psp[:], nc_f[:])           # ln guard
        nc.scalar.activation(lnpsp[:], u[:], AF.Ln)
        nc.vector.tensor_mul(u[:], psp[:], lnpsp[:])
        nc.vector.tensor_sub(ent[:], ent[:], u[:])
        nc.vector.tensor_scalar_mul(ent[:], ent[:], GAMMA)
        nc.vector.tensor_scalar_mul(u[:], psp[:], -GAMMA * lg)
        nc.vector.tensor_add(ent[:], ent[:], u[:])
        nc.vector.tensor_scalar(out=ent[:], in0=ent[:], scalar1=GAMMA * lg,
                                scalar2=None, op0=OP.add)
        nc.vector.tensor_scalar_mul(tsp[:], psp[:], GAMMA)
        nc.vector.tensor_scalar(out=tsp[:], in0=tsp[:], scalar1=1.0 - GAMMA,
                                scalar2=None, op0=OP.add)
        nc.scalar.activation(u[:], tsp[:], AF.Ln)
        nc.vector.tensor_mul(u[:], u[:], tsp[:])
        nc.vector.tensor_mul(w[:], nc_f[:], u[:])
        nc.vector.tensor_sub(u[:], u[:], w[:])                # (1-nc)*...
        nc.vector.tensor_add(ent[:], ent[:], u[:])
        nc.vector.tensor_add(acc2[:], acc2[:], ent[:])        # row_txt
        rtxt = sm.tile([P, 1], FT, tag="rtxt")
        nc.vector.tensor_reduce(out=rtxt[:], in_=acc2[:], axis=AX.X, op=OP.add)
        tree_red(rtxt)

        # ---------------- row_L finish (all local) ----------------
        # row_L = EntL - s*(g*TSLa + 0.2*ncsno) + (m2L + lnZ2L)*(g + 0.2*nc_num)
        t11a = sm.tile([1, 1], FT, tag="t11a")
        nc.vector.tensor_scalar_mul(t11a[:], TSLa11[:], GAMMA)
        nc.vector.tensor_scalar_mul(r11[:], ncsno11[:], 1.0 - GAMMA)
        nc.vector.tensor_add(t11a[:], t11a[:], r11[:])
        nc.vector.tensor_mul(t11a[:], t11a[:], s11[:])
        nc.scalar.activation(r11[:], Z2L11[:], AF.Ln)
        nc.vector.tensor_add(r11[:], r11[:], m2L11[:])
        nc.vector.tensor_scalar_mul(r11[:], r11[:],
                                    GAMMA + (1.0 - GAMMA) * nc_num)
        rowL = sm.tile([1, 1], FT, tag="rowL")
        nc.vector.tensor_sub(rowL[:], entL11[:], t11a[:])
        nc.vector.tensor_add(rowL[:], rowL[:], r11[:])

        # ---------------- final output (no collective) ----------------
        res = sm.tile([1, 2], FT, tag="res")
        li = sm.tile([1, 1], FT, tag="li")
        nc.vector.tensor_scalar_mul(li[:], rimg[0:1, 0:1], 0.5 / N)
        lt = sm.tile([1, 1], FT, tag="lt")
        nc.vector.tensor_add(lt[:], rtxt[0:1, 0:1], rowL[:])
        nc.vector.tensor_scalar_mul(lt[:], lt[:], 0.5 / (N + 1))
        nc.vector.tensor_add(res[0:1, 0:1], li[:], lt[:])
        nc.vector.tensor_scalar_mul(res[0:1, 1:2], hng11[:], 1.0 / N)
        nc.sync.dma_start(outs["out"][:], res[:])

        if dbg:
            for name, t_ in (("tau_o", tau_o), ("nc_o", nc_o),
                             ("a1_f", a1_f), ("b1_f", b1_f), ("b2_f", b2_f),
                             ("R1_f", R1_f), ("R2_f", R2_f), ("M1_f", M1_f),
                             ("T2_f", T2_f), ("T3_f", T3_f), ("m_f", m_f),
                             ("Zd_f", Zd_f), ("m2_f", m2_f), ("Zd2_f", Zd2_f),
                             ("acc", acc), ("acc2", acc2), ("rA_o", rA_o)):
                nc.sync.dma_start(outs[name][:], t_[:])
            nc.sync.dma_start(outs["kth"][:], kth[:])
            nc.sync.dma_start(outs["sc"][:], sc4[0:1, 0:1])


# ============================================================================
# runner: full-input kernel entry point (N=4096, 8 NeuronCores)
# ============================================================================

_NC = 8
_N = 4096
_CFG = cfg_for(_N, ncores=_NC)
_CACHE = {}
LAST_EXEC_NS = None


def _build_nc(dbg=False):
    import concourse.bacc as bacc
    nc = bacc.Bacc("TRN2", target_bir_lowering=False, debug=False,
                   num_devices=_NC, enable_asserts=False)
    R = _CFG["R"]
    CH = _CFG["CH"]
    shapes = {
        "eit_own": ((D, R), FR), "ett_own": ((D, R), FR),
        "eit_full": ((D, _N), FR), "ett_full": ((D, _N), FR),
        "ei_r": ((R, D), FT), "et_r": ((R, D), FT), "en_r": ((R, D), FT),
        "lscale": ((1, 1), FT), "own_mask": ((1, CH), FT),
    }
    ins = {k: nc.dram_tensor(k, sh, dt, kind="ExternalInput").ap()
           for k, (sh, dt) in shapes.items()}
    outs = {"out": nc.dram_tensor("out", (1, 2), FT,
                                  kind="ExternalOutput").ap()}
    if dbg:
        G = _CFG["G"]
        for nm in ("tau_o", "nc_o", "rA_o"):
            outs[nm] = nc.dram_tensor(nm, (P, G), FT,
                                      kind="ExternalOutput").ap()
        for nm in ("a1_f", "b1_f", "b2_f", "R1_f", "R2_f", "M1_f", "T2_f",
                   "T3_f", "m_f", "Zd_f", "m2_f", "Zd2_f", "acc", "acc2"):
            outs[nm] = nc.dram_tensor(nm, (P, CH), FT,
                                      kind="ExternalOutput").ap()
        outs["kth"] = nc.dram_tensor("kth", (1, 2), FT,
                                     kind="ExternalOutput").ap()
        outs["sc"] = nc.dram_tensor("sc", (1, 1), FT,
                                    kind="ExternalOutput").ap()
    with tile.TileContext(nc) as tc:
        build_kernel(tc, outs, ins, _CFG, dbg=dbg)
    nc.compile()
    return nc


def kernel(image_embeds, text_embeds, text_no_embeds, logit_scale):
    """Full (unsharded) inputs -> (loss_ul, loss_op), computed on 8 trn2
    NeuronCores (data-parallel rows, 4 collectives, T=1 Sinkhorn)."""
    global LAST_EXEC_NS
    from concourse.bass_utils import run_bass_kernel_spmd

    Ei = np.ascontiguousarray(np.asarray(image_embeds, dtype=np.float32))
    Et = np.ascontiguousarray(np.asarray(text_embeds, dtype=np.float32))
    En = np.ascontiguousarray(np.asarray(text_no_embeds, dtype=np.float32))
    ls = np.float32(np.asarray(logit_scale, dtype=np.float32))

    if "nc" not in _CACHE:
        _CACHE["nc"] = _build_nc()
    nc = _CACHE["nc"]

    per_core = shard_inputs(Ei, Et, En, ls, _CFG)
    res = run_bass_kernel_spmd(nc, per_core, core_ids=list(range(_NC)),
                               trace=False)
    out = res.results[0]["out"]
    return (np.float32(out[0, 0]), np.float32(out[0, 1]))


# ---------------------------------------------------------------------------
# timing helper (not used by the grading path): persistent jitted executable
# with device-resident inputs; per-iteration wall time measures NEFF exec +
# dispatch. Pair with a dummy kernel of identical structure to subtract the
# dispatch floor.
# ---------------------------------------------------------------------------

def _make_exec(nc, in_maps):
    import jax
    import concourse.mybir as _mybir
    from jax.sharding import Mesh, PartitionSpec, NamedSharding
    from jax.experimental.shard_map import shard_map
    from concourse import bass2jax as _b2j

    n_cores = len(in_maps)
    part_name = nc.partition_id_tensor.name if nc.partition_id_tensor else None
    in_names, out_names, out_avals, zero_outs = [], [], [], []
    for alloc in nc.m.functions[0].allocations:
        if not isinstance(alloc, _mybir.MemoryLocationSet):
            continue
        name = alloc.memorylocations[0].name
        if alloc.kind == "ExternalInput":
            if name != part_name:
                in_names.append(name)
        elif alloc.kind == "ExternalOutput":
            out_names.append(name)
            shape = tuple(alloc.tensor_shape)
            dtype = _mybir.dt.np(alloc.dtype)
            out_avals.append(jax.core.ShapedArray(shape, dtype))
            zero_outs.append(np.zeros(shape, dtype))
    n_params = len(in_names)
    n_outs = len(out_avals)
    all_names = in_names + out_names
    if part_name is not None:
        all_names = all_names + [part_name]

    def _body(*args):
        operands = list(args)
        if part_name is not None:
            operands.append(_b2j.partition_id_tensor())
        outs = _b2j._bass_exec_p.bind(
            *operands, out_avals=tuple(out_avals), in_names=tuple(all_names),
            out_names=tuple(out_names), lowering_input_output_aliases=(),
            sim_require_finite=True, sim_require_nnan=True, nc=nc)
        return tuple(outs)

    devices = jax.devices()[:n_cores]
    mesh = Mesh(np.asarray(devices), ("core",))
    in_specs = (PartitionSpec("core"),) * (n_params + n_outs)
    out_specs = (PartitionSpec("core"),) * n_outs
    sharded = jax.jit(shard_map(_body, mesh=mesh, in_specs=in_specs,
                                out_specs=out_specs, check_rep=False),
                      donate_argnums=tuple(range(n_params, n_params + n_outs)),
                      keep_unused=True)
    sh = NamedSharding(mesh, PartitionSpec("core"))
    dev_in = [jax.device_put(
        np.concatenate([np.asarray(m[name]) for m in in_maps], axis=0), sh)
        for name in in_names]

    def run_once():
        zeros = [np.zeros((n_cores * z.shape[0], *z.shape[1:]), z.dtype)
                 for z in zero_outs]
        outs = sharded(*dev_in, *zeros)
        for o in outs:
            o.block_until_ready()
        return outs

    return run_once


def timed_ns(iters=5):
    """Median wall ns of one 8-core NEFF dispatch+exec of the main kernel,
    and of a trivial dummy kernel (for dispatch-floor calibration)."""
    import time
    import concourse.bacc as bacc
    import concourse.tile as tile_

    rng = np.random.default_rng(0)
    Ei = rng.standard_normal((_N, D)).astype(np.float32)
    Ei /= np.linalg.norm(Ei, axis=1, keepdims=True)
    Et = rng.standard_normal((_N, D)).astype(np.float32)
    Et /= np.linalg.norm(Et, axis=1, keepdims=True)
    En = rng.standard_normal((_N, D)).astype(np.float32)
    En /= np.linalg.norm(En, axis=1, keepdims=True)
    if "nc" not in _CACHE:
        _CACHE["nc"] = _build_nc()
    per_core = shard_inputs(Ei, Et, En, np.float32(1.0), _CFG)

    run_main = _make_exec(_CACHE["nc"], per_core)

    # dummy: same I/O contract graph shape, near-zero work
    ncd = bacc.Bacc("TRN2", target_bir_lowering=False, debug=False,
                    num_devices=_NC, enable_asserts=False)
    din = ncd.dram_tensor("x", (1, 2), FT, kind="ExternalInput").ap()
    dout = ncd.dram_tensor("out", (1, 2), FT, kind="ExternalOutput").ap()
    with tile_.TileContext(ncd) as tcd:
        with tcd.tile_pool(name="p", bufs=1) as pool:
            t = pool.tile([1, 2], FT, name="t")
            ncd.sync.dma_start(t[:], din[:])
            ncd.sync.dma_start(dout[:], t[:])
    ncd.compile()
    run_dummy = _make_exec(ncd, [{"x": np.zeros((1, 2), np.float32)}
                                 for _ in range(_NC)])

    def med(fn):
        ts = []
        fn()  # warm
        for _ in range(iters):
            t0 = time.perf_counter()
            fn()
            ts.append(time.perf_counter() - t0)
        ts.sort()
        return ts[len(ts) // 2] * 1e9

    t_dummy = med(run_dummy)
    t_main = med(run_main)
    return t_main, t_dummy
